# revision 28
# baseline (speedup 1.0000x reference)
"""Patch-embedding kernel for Trainium2, data-parallel over batch on 8 NeuronCores.

Reference computation (per image):
  patches = im2col(image, 16x16)            # [196, 768]
  out = gelu(patches @ W + b, exact)        # [196, 768]

Sharding: batch 64 -> 8 images per core; host concatenates per-core outputs.

Layout strategy: im2col is a pure permutation for stride-16 non-overlapping
patches, so the HOST performs im2col + transpose + bf16 cast and uploads
X^T in per-m-block slabs xb[p, kc, j] = X[moff+j, 128*kc + p]. Every device
load is then fully contiguous per partition (3-4.7 KB packets, ~355 GB/s);
the previous single [p, kc, M] layout produced 392-784 B packets on the
m-sliced block loads and streamed at only ~215-280 GB/s.

Matmul orientation: transposed output. For each 128-wide n-chunk,
  psum[n, m] = sum_kc W[k, n].T @ X^T[k, m]
with W chunks as the stationary operand (natural layout, uploaded
pre-chunked) and X^T as the bf16 moving operand. Benefits:
  - no bias matmuls: bias is per-PARTITION in this orientation, applied for
    free by ScalarE as gelu(psum + bias[p]) during the PSUM->SBUF pass
  - output stored bf16 per-block [p, n6, j] slabs: ONE fat store per block
    (4.7 KB/partition packets) instead of six thin ones; host unscrambles.

Per-core schedule (m-blocks of 392/392/392/196/196):
  - The DMA ring processes in-flight descriptors CONCURRENTLY with
    per-packet round-robin (bandwidth share ~ packet size), and holds
    ~8 descriptors. So only W + x0 + x1 are issued up front, in exact
    consumption order; x2..x4 are "token"-gated (a 1-element copy off
    the previous block's tile gives a real dependency the scheduler
    can't hoist) so they join the ring only after the early set drains.
  - 11 matmuls on a zeroed tile bridge the PE from the entry barrier to
    the first real matmul so the HAM clock gate (cold 1.2 GHz -> warm
    2.4 GHz after ~3-6 us of sustained busy; gaps >~1 us reset it)
    steps up while the stream is still delivering. They accumulate into
    ps0[0], whose first real matmul is start=True anyway, so no extra
    PSUM bank is held (block 1 needs 6 of the 8 banks).
  - Block 1 is 392 wide and runs kc-outer with all 6 n-chunk PSUM
    groups live: each W/X chunk-pair arrival yields 2 us (cold) of
    matmuls, more than the inter-arrival gap, so the PE stays
    continuously busy from warmup through block 1 (protects the clock
    gate). Later blocks run n6-outer, one PSUM bank per group.
  - ScalarE applies exact GELU (+ per-partition bias) PSUM->SBUF bf16
    into a per-block [p, n6, j] tile; one store per block overlaps the
    next block's compute. The LAST block stores n6 0:5 early on the
    idle sync ring and n6=5 from the scalar ring itself, so the final
    exposed chain is matmul -> GELU -> one tiny store.
Steady-state matmul cadence runs at the N-cycle streaming floor
(163 ns for N=392 bf16, LDWEIGHTS fully hidden); measured HW exec
41.0 us (from 44.3 us baseline), ~1.5 us above the practical floor of
preamble (~8.7 to first DMA byte) + wire-gated start (~2.5) + 23.5 PE
+ tail (~1.1) + teardown (~3.2).
"""

import numpy as np
import ml_dtypes

import concourse.bass as bass
import concourse.tile as tile
import concourse.mybir as mybir
from concourse import bacc
from concourse.bass_utils import run_bass_kernel_spmd

P = 16
D = 768
B, H, W, C = 64, 224, 224, 3
NH = NW = 14
NPATCH = NH * NW            # 196
K = P * P * C               # 768
NCORES = 8
BPC = B // NCORES           # 8 images per core
M = BPC * NPATCH            # 1568 output rows per core
KC = K // 128               # 6 k-chunks
NC6 = D // 128              # 6 n-chunks
MBLOCKS = (392, 392, 392, 196, 196)
MOFF = (0, 392, 784, 1176, 1372)

_BF16 = mybir.dt.bfloat16
_F32 = mybir.dt.float32


def _build():
    nc = bacc.Bacc("TRN2", target_bir_lowering=False, debug=False,
                   num_devices=NCORES)
    # Host-prepared layouts (see _run): all reads/writes contiguous.
    xbs = [nc.dram_tensor(f"x{i}", [128, KC, mw], _BF16,
                          kind="ExternalInput").ap()
           for i, mw in enumerate(MBLOCKS)]
    w = nc.dram_tensor("w", [128, KC, D], _BF16, kind="ExternalInput").ap()
    bias = nc.dram_tensor("bias", [128, NC6], _F32, kind="ExternalInput").ap()
    # Transposed block outputs o{i}[p, n6, j] = result[moff+j, 128*n6 + p].
    obs = [nc.dram_tensor(f"o{i}", [128, NC6, mw], _BF16,
                          kind="ExternalOutput").ap()
           for i, mw in enumerate(MBLOCKS)]

    with tile.TileContext(nc) as tc:
        _body(tc, xbs, w, bias, obs)
    nc.compile()
    return nc


def _body(tc, xbs, w, bias, obs):
    import contextlib
    ctx = contextlib.ExitStack()
    with ctx:
        nc = tc.nc
        singles = ctx.enter_context(tc.tile_pool(name="singles", bufs=1))
        pspool = ctx.enter_context(tc.tile_pool(name="ps", bufs=8, space="PSUM"))

        # Loads on the sync ring in exact consumption order; bias on the
        # scalar ring (only needed by the first activation).
        bias_sb = singles.tile([128, NC6], _F32)
        nc.scalar.dma_start(out=bias_sb[:], in_=bias[:])
        w_sb = singles.tile([128, KC, D], _BF16)
        x_sb = [singles.tile([128, KC, mw], _BF16, name=f"x{i}")
                for i, mw in enumerate(MBLOCKS)]
        # The DMA ring processes all in-flight descriptors CONCURRENTLY
        # (round-robin packets), so a descriptor's completion sem fires
        # roughly at start + size * n_live / BW. Only block 1 + block 2
        # dependencies are issued up front (small, early pieces first);
        # x2..x4 are emitted after block 1's store below, so they join
        # the ring only once the early set has drained.
        nc.sync.dma_start(out=w_sb[:, 0:2, :], in_=w[:, 0:2, :])
        nc.sync.dma_start(out=x_sb[0][:, 0:2, :], in_=xbs[0][:, 0:2, :])
        nc.sync.dma_start(out=w_sb[:, 2:4, :], in_=w[:, 2:4, :])
        nc.sync.dma_start(out=x_sb[0][:, 2:4, :], in_=xbs[0][:, 2:4, :])
        nc.sync.dma_start(out=w_sb[:, 4:6, :], in_=w[:, 4:6, :])
        nc.sync.dma_start(out=x_sb[0][:, 4:6, :], in_=xbs[0][:, 4:6, :])
        # Late block loads, self-paced: a 1-element "token" copy gives
        # each late descriptor a REAL data dependency (scheduler can't
        # hoist it) on an earlier transfer, so it only joins the ring
        # once the wire has drained. With per-packet round-robin, a live
        # 4.7KB-packet block load would otherwise starve the 1.5-3KB
        # critical descriptors. x1 is gated on x0's FIRST chunk (fires
        # ~11us, before the wire drains, so it never idles -- gating x1
        # on x0's last chunk was tried and loses ~1.5us to an idle wire
        # + b2 stall); x2..x4 chain on the previous block's transfer.
        # The token writes are overwritten by the DMA.
        nc.vector.tensor_copy(x_sb[1][0:1, 0, 0:1],
                              x_sb[0][0:1, 1, MBLOCKS[0] - 1:MBLOCKS[0]])
        nc.sync.dma_start(out=x_sb[1][:], in_=xbs[1][:])
        nc.vector.tensor_copy(x_sb[2][0:1, 0, 0:1],
                              x_sb[1][0:1, KC - 1, MBLOCKS[1] - 1:MBLOCKS[1]])
        nc.sync.dma_start(out=x_sb[2][:], in_=xbs[2][:])
        nc.vector.tensor_copy(x_sb[3][0:1, 0, 0:1],
                              x_sb[2][0:1, KC - 1, MBLOCKS[2] - 1:MBLOCKS[2]])
        nc.sync.dma_start(out=x_sb[3][:], in_=xbs[3][:])
        nc.vector.tensor_copy(x_sb[4][0:1, 0, 0:1],
                              x_sb[3][0:1, KC - 1, MBLOCKS[3] - 1:MBLOCKS[3]])
        nc.sync.dma_start(out=x_sb[4][:], in_=xbs[4][:])

        # First m-block: kc-outer with all 6 n-chunk PSUM groups live.
        ps0 = [pspool.tile([128, 512], _F32, tag="ps", name=f"ps0_{i}")
               for i in range(NC6)]

        # PE warmup bridging the entry barrier to the first real matmul
        # (see module docstring).
        zeros = singles.tile([128, 384], _BF16)
        nc.vector.memset(zeros[:], 0.0)
        for _ in range(11):
            nc.tensor.matmul(ps0[0][:, :384], zeros[:, :128], zeros[:],
                             start=True, stop=True)

        B0 = MBLOCKS[0]
        for kc in range(KC):
            for n6 in range(NC6):
                nc.tensor.matmul(ps0[n6][:, :B0],
                                 w_sb[:, kc, n6 * 128:(n6 + 1) * 128],
                                 x_sb[0][:, kc, :],
                                 start=(kc == 0), stop=(kc == KC - 1))
        o_t = singles.tile([128, NC6, B0], _BF16, name="ot0")
        for n6 in range(NC6):
            nc.scalar.activation(o_t[:, n6, :], ps0[n6][:, :B0],
                                 mybir.ActivationFunctionType.Gelu,
                                 bias=bias_sb[:, n6:n6 + 1])
        nc.sync.dma_start(out=obs[0][:], in_=o_t[:])

        # Remaining m-blocks: data is resident (or lands just ahead);
        # n6-outer keeps the activation pipeline finely paced.
        last = len(MBLOCKS) - 1
        for mb in range(1, len(MBLOCKS)):
            mw = MBLOCKS[mb]
            o_t = singles.tile([128, NC6, mw], _BF16, name=f"ot{mb}")
            for n6 in range(NC6):
                ps = pspool.tile([128, 512], _F32, tag="ps")
                for kc in range(KC):
                    nc.tensor.matmul(ps[:, :mw],
                                     w_sb[:, kc, n6 * 128:(n6 + 1) * 128],
                                     x_sb[mb][:, kc, :],
                                     start=(kc == 0), stop=(kc == KC - 1))
                nc.scalar.activation(o_t[:, n6, :], ps[:, :mw],
                                     mybir.ActivationFunctionType.Gelu,
                                     bias=bias_sb[:, n6:n6 + 1])
                if mb == last and n6 in (1, 3, 4):
                    # drain the last block incrementally as its GELUs
                    # complete, so only ~100 KB of output (n6 4:6)
                    # remains in flight after the final matmul. All on
                    # the warm multi-engine sync ring (a lone small
                    # descriptor on the scalar ring gets ONE SDMA
                    # engine, ~45 GB/s -- measured 1.1 us for 50 KB).
                    lo = n6 - 1 if n6 != 4 else 4
                    nc.sync.dma_start(out=obs[mb][:, lo:n6 + 1, :],
                                      in_=o_t[:, lo:n6 + 1, :])
            if mb == last:
                nc.sync.dma_start(out=obs[mb][:, NC6 - 1:NC6, :],
                                  in_=o_t[:, NC6 - 1:NC6, :])
            else:
                nc.sync.dma_start(out=obs[mb][:], in_=o_t[:])


_NC_CACHE = {}


def _get_nc():
    if "nc" not in _NC_CACHE:
        _NC_CACHE["nc"] = _build()
    return _NC_CACHE["nc"]


def _prep_core_inputs(image, W_proj, b_proj):
    """Host-side layout prep: im2col + transpose + bf16, all permutations."""
    image = np.asarray(image, dtype=np.float32)
    assert image.shape == (B, H, W, C)
    img_bf = image.astype(ml_dtypes.bfloat16)
    # im2col (row-major patch order, matching the reference)
    pat = img_bf.reshape(B, NH, P, NW, P, C).transpose(0, 1, 3, 2, 4, 5)
    pat = np.ascontiguousarray(pat).reshape(B, NPATCH, K)

    w_bf = np.asarray(W_proj, dtype=np.float32).astype(ml_dtypes.bfloat16)
    w_dev = np.ascontiguousarray(w_bf.reshape(KC, 128, D).transpose(1, 0, 2))
    b_dev = np.ascontiguousarray(
        np.asarray(b_proj, dtype=np.float32).reshape(NC6, 128).T)

    in_maps = []
    for c in range(NCORES):
        x = pat[c * BPC:(c + 1) * BPC].reshape(M, K)
        # xt[p, kc, m] = x[m, 128*kc + p]
        xt = x.reshape(M, KC, 128).transpose(2, 1, 0)
        im = {"w": w_dev, "bias": b_dev}
        for i, mw in enumerate(MBLOCKS):
            im[f"x{i}"] = np.ascontiguousarray(
                xt[:, :, MOFF[i]:MOFF[i] + mw])
        in_maps.append(im)
    return in_maps


def _run(image, W_proj, b_proj, **spmd_kwargs):
    spmd_kwargs.pop("transpose_mode", None)
    in_maps = _prep_core_inputs(image, W_proj, b_proj)
    nc = _get_nc()
    res = run_bass_kernel_spmd(nc, in_maps, core_ids=list(range(NCORES)),
                               **spmd_kwargs)
    # block layout o{i}[p, n6, j] -> [moff+j, 128*n6+p] -> rows of [M, D]
    outs = []
    for c in range(NCORES):
        rows = np.empty((M, D), dtype=np.float32)
        for i, mw in enumerate(MBLOCKS):
            blk = res.results[c][f"o{i}"]  # [128, NC6, mw] bf16
            rows[MOFF[i]:MOFF[i] + mw] = (
                blk.transpose(2, 1, 0).reshape(mw, D).astype(np.float32))
        outs.append(rows.reshape(BPC, NPATCH, D))
    full = np.concatenate(outs, axis=0)
    return full, res


def kernel(image, W_proj, b_proj):
    full, _ = _run(image, W_proj, b_proj)
    return full


# revision 30
# speedup vs baseline: 1.0688x; 1.0688x over previous
"""Patch-embedding kernel for Trainium2, data-parallel over batch on 8 NeuronCores.

Reference computation (per image):
  patches = im2col(image, 16x16)            # [196, 768]
  out = gelu(patches @ W + b, exact)        # [196, 768]

Sharding: batch 64 -> 8 images per core; host concatenates per-core outputs.

Layout strategy: im2col is a pure permutation for stride-16 non-overlapping
patches, so the HOST performs im2col + transpose + bf16 cast and uploads
X^T in per-m-block slabs xb[p, kc, j] = X[moff+j, 128*kc + p]. Every device
load is then fully contiguous per partition (3-4.7 KB packets, ~355 GB/s);
the previous single [p, kc, M] layout produced 392-784 B packets on the
m-sliced block loads and streamed at only ~215-280 GB/s.

Matmul orientation: transposed output. For each 128-wide n-chunk,
  psum[n, m] = sum_kc W[k, n].T @ X^T[k, m]
with W chunks as the stationary operand (natural layout, uploaded
pre-chunked) and X^T as the bf16 moving operand. Benefits:
  - no bias matmuls: bias is per-PARTITION in this orientation, applied for
    free by ScalarE as gelu(psum + bias[p]) during the PSUM->SBUF pass
  - output stored bf16 per-block [p, n6, j] slabs: ONE fat store per block
    (4.7 KB/partition packets) instead of six thin ones; host unscrambles.

Per-core schedule (m-blocks of 392/392/392/196/196):
  - The DMA ring processes in-flight descriptors CONCURRENTLY with
    per-packet round-robin (bandwidth share ~ packet size), and holds
    ~8 descriptors. So only W + x0 + x1 are issued up front, in exact
    consumption order; x2..x4 are "token"-gated (a 1-element copy off
    the previous block's tile gives a real dependency the scheduler
    can't hoist) so they join the ring only after the early set drains.
  - 11 matmuls on a zeroed tile bridge the PE from the entry barrier to
    the first real matmul so the HAM clock gate (cold 1.2 GHz -> warm
    2.4 GHz after ~3-6 us of sustained busy; gaps >~1 us reset it)
    steps up while the stream is still delivering. They accumulate into
    ps0[0], whose first real matmul is start=True anyway, so no extra
    PSUM bank is held (block 1 needs 6 of the 8 banks).
  - Block 1 is 392 wide and runs kc-outer with all 6 n-chunk PSUM
    groups live: each W/X chunk-pair arrival yields 2 us (cold) of
    matmuls, more than the inter-arrival gap, so the PE stays
    continuously busy from warmup through block 1 (protects the clock
    gate). Later blocks run n6-outer, one PSUM bank per group.
  - ScalarE applies exact GELU (+ per-partition bias) PSUM->SBUF bf16
    into a per-block [p, n6, j] tile; one store per block overlaps the
    next block's compute. The LAST block stores n6 0:5 early on the
    idle sync ring and n6=5 from the scalar ring itself, so the final
    exposed chain is matmul -> GELU -> one tiny store.
Steady-state matmul cadence runs at the N-cycle streaming floor
(163 ns for N=392 bf16, LDWEIGHTS fully hidden); measured HW exec
41.0 us (from 44.3 us baseline), ~1.5 us above the practical floor of
preamble (~8.7 to first DMA byte) + wire-gated start (~2.5) + 23.5 PE
+ tail (~1.1) + teardown (~3.2).
"""

import numpy as np
import ml_dtypes

import concourse.bass as bass
import concourse.tile as tile
import concourse.mybir as mybir
from concourse import bacc
from concourse.bass_utils import run_bass_kernel_spmd

P = 16
D = 768
B, H, W, C = 64, 224, 224, 3
NH = NW = 14
NPATCH = NH * NW            # 196
K = P * P * C               # 768
NCORES = 8
BPC = B // NCORES           # 8 images per core
M = BPC * NPATCH            # 1568 output rows per core
KC = K // 128               # 6 k-chunks
NC6 = D // 128              # 6 n-chunks
MBLOCKS = (392, 392, 392, 196, 196)
MOFF = (0, 392, 784, 1176, 1372)

_BF16 = mybir.dt.bfloat16
_F32 = mybir.dt.float32


def _build():
    nc = bacc.Bacc("TRN2", target_bir_lowering=False, debug=False,
                   num_devices=NCORES)
    # Host-prepared layouts (see _run): all reads/writes contiguous.
    xbs = [nc.dram_tensor(f"x{i}", [128, KC, mw], _BF16,
                          kind="ExternalInput").ap()
           for i, mw in enumerate(MBLOCKS)]
    w = nc.dram_tensor("w", [128, KC, D], _BF16, kind="ExternalInput").ap()
    bias = nc.dram_tensor("bias", [128, NC6], _F32, kind="ExternalInput").ap()
    # Transposed block outputs o{i}[p, n6, j] = result[moff+j, 128*n6 + p].
    obs = [nc.dram_tensor(f"o{i}", [128, NC6, mw], _BF16,
                          kind="ExternalOutput").ap()
           for i, mw in enumerate(MBLOCKS)]

    with tile.TileContext(nc) as tc:
        _body(tc, xbs, w, bias, obs)
    nc.compile()
    return nc


def _body(tc, xbs, w, bias, obs):
    import contextlib
    ctx = contextlib.ExitStack()
    with ctx:
        nc = tc.nc
        singles = ctx.enter_context(tc.tile_pool(name="singles", bufs=1))
        pspool = ctx.enter_context(tc.tile_pool(name="ps", bufs=8, space="PSUM"))

        # Loads on the sync ring in exact consumption order; bias on the
        # scalar ring (only needed by the first activation).
        bias_sb = singles.tile([128, NC6], _F32)
        nc.scalar.dma_start(out=bias_sb[:], in_=bias[:])
        w_sb = singles.tile([128, KC, D], _BF16)
        x_sb = [singles.tile([128, KC, mw], _BF16, name=f"x{i}")
                for i, mw in enumerate(MBLOCKS)]
        # The DMA ring processes all in-flight descriptors CONCURRENTLY
        # (round-robin packets), so a descriptor's completion sem fires
        # roughly at start + size * n_live / BW. Only block 1 + block 2
        # dependencies are issued up front (small, early pieces first);
        # x2..x4 are emitted after block 1's store below, so they join
        # the ring only once the early set has drained.
        nc.sync.dma_start(out=w_sb[:, 0:2, :], in_=w[:, 0:2, :])
        nc.sync.dma_start(out=x_sb[0][:, 0:2, :], in_=xbs[0][:, 0:2, :])
        nc.sync.dma_start(out=w_sb[:, 2:4, :], in_=w[:, 2:4, :])
        nc.sync.dma_start(out=x_sb[0][:, 2:4, :], in_=xbs[0][:, 2:4, :])
        nc.sync.dma_start(out=w_sb[:, 4:6, :], in_=w[:, 4:6, :])
        nc.sync.dma_start(out=x_sb[0][:, 4:6, :], in_=xbs[0][:, 4:6, :])
        nc.sync.dma_start(out=x_sb[1][:], in_=xbs[1][:])
        # Late block loads, self-paced: a 1-element "token" copy gives
        # each late descriptor a REAL data dependency (scheduler can't
        # hoist it) on an earlier transfer, so it only joins the ring
        # once the wire has drained: x2 after x1 lands, x3 after x2,
        # x4 after x3. The token writes are overwritten by the DMA.
        # With per-packet round-robin, a live 4.7KB-packet block load
        # would otherwise starve the 1.5-3KB critical descriptors.
        # (Gating x1 too -- on x0's last OR first chunk -- was tried;
        # neither beat keeping x1 upfront.)
        nc.vector.tensor_copy(x_sb[2][0:1, 0, 0:1],
                              x_sb[1][0:1, KC - 1, MBLOCKS[1] - 1:MBLOCKS[1]])
        nc.sync.dma_start(out=x_sb[2][:], in_=xbs[2][:])
        nc.vector.tensor_copy(x_sb[3][0:1, 0, 0:1],
                              x_sb[2][0:1, KC - 1, MBLOCKS[2] - 1:MBLOCKS[2]])
        nc.sync.dma_start(out=x_sb[3][:], in_=xbs[3][:])
        nc.vector.tensor_copy(x_sb[4][0:1, 0, 0:1],
                              x_sb[3][0:1, KC - 1, MBLOCKS[3] - 1:MBLOCKS[3]])
        nc.sync.dma_start(out=x_sb[4][:], in_=xbs[4][:])

        # First m-block: kc-outer with all 6 n-chunk PSUM groups live.
        ps0 = [pspool.tile([128, 512], _F32, tag="ps", name=f"ps0_{i}")
               for i in range(NC6)]

        # PE warmup bridging the entry barrier to the first real matmul
        # (see module docstring).
        zeros = singles.tile([128, 384], _BF16)
        nc.vector.memset(zeros[:], 0.0)
        for _ in range(11):
            nc.tensor.matmul(ps0[0][:, :384], zeros[:, :128], zeros[:],
                             start=True, stop=True)

        B0 = MBLOCKS[0]
        for kc in range(KC):
            for n6 in range(NC6):
                nc.tensor.matmul(ps0[n6][:, :B0],
                                 w_sb[:, kc, n6 * 128:(n6 + 1) * 128],
                                 x_sb[0][:, kc, :],
                                 start=(kc == 0), stop=(kc == KC - 1))
        o_t = singles.tile([128, NC6, B0], _BF16, name="ot0")
        for n6 in range(NC6):
            nc.scalar.activation(o_t[:, n6, :], ps0[n6][:, :B0],
                                 mybir.ActivationFunctionType.Gelu,
                                 bias=bias_sb[:, n6:n6 + 1])
        nc.sync.dma_start(out=obs[0][:], in_=o_t[:])

        # Remaining m-blocks: data is resident (or lands just ahead);
        # n6-outer keeps the activation pipeline finely paced.
        last = len(MBLOCKS) - 1
        for mb in range(1, len(MBLOCKS)):
            mw = MBLOCKS[mb]
            o_t = singles.tile([128, NC6, mw], _BF16, name=f"ot{mb}")
            for n6 in range(NC6):
                ps = pspool.tile([128, 512], _F32, tag="ps")
                for kc in range(KC):
                    nc.tensor.matmul(ps[:, :mw],
                                     w_sb[:, kc, n6 * 128:(n6 + 1) * 128],
                                     x_sb[mb][:, kc, :],
                                     start=(kc == 0), stop=(kc == KC - 1))
                nc.scalar.activation(o_t[:, n6, :], ps[:, :mw],
                                     mybir.ActivationFunctionType.Gelu,
                                     bias=bias_sb[:, n6:n6 + 1])
                if mb == last and n6 == NC6 - 2:
                    # store the bulk early on the (idle) sync ring so
                    # its issue doesn't block the final GELU on scalar;
                    # only n6=5 stays exposed. (Finer incremental drain
                    # and moving the final store to sync were both
                    # tried and measured slower.)
                    nc.sync.dma_start(out=obs[mb][:, 0:NC6 - 1, :],
                                      in_=o_t[:, 0:NC6 - 1, :])
            if mb == last:
                nc.scalar.dma_start(out=obs[mb][:, NC6 - 1:NC6, :],
                                    in_=o_t[:, NC6 - 1:NC6, :])
            else:
                nc.sync.dma_start(out=obs[mb][:], in_=o_t[:])


_NC_CACHE = {}


def _get_nc():
    if "nc" not in _NC_CACHE:
        _NC_CACHE["nc"] = _build()
    return _NC_CACHE["nc"]


def _prep_core_inputs(image, W_proj, b_proj):
    """Host-side layout prep: im2col + transpose + bf16, all permutations."""
    image = np.asarray(image, dtype=np.float32)
    assert image.shape == (B, H, W, C)
    img_bf = image.astype(ml_dtypes.bfloat16)
    # im2col (row-major patch order, matching the reference)
    pat = img_bf.reshape(B, NH, P, NW, P, C).transpose(0, 1, 3, 2, 4, 5)
    pat = np.ascontiguousarray(pat).reshape(B, NPATCH, K)

    w_bf = np.asarray(W_proj, dtype=np.float32).astype(ml_dtypes.bfloat16)
    w_dev = np.ascontiguousarray(w_bf.reshape(KC, 128, D).transpose(1, 0, 2))
    b_dev = np.ascontiguousarray(
        np.asarray(b_proj, dtype=np.float32).reshape(NC6, 128).T)

    in_maps = []
    for c in range(NCORES):
        x = pat[c * BPC:(c + 1) * BPC].reshape(M, K)
        # xt[p, kc, m] = x[m, 128*kc + p]
        xt = x.reshape(M, KC, 128).transpose(2, 1, 0)
        im = {"w": w_dev, "bias": b_dev}
        for i, mw in enumerate(MBLOCKS):
            im[f"x{i}"] = np.ascontiguousarray(
                xt[:, :, MOFF[i]:MOFF[i] + mw])
        in_maps.append(im)
    return in_maps


def _run(image, W_proj, b_proj, **spmd_kwargs):
    spmd_kwargs.pop("transpose_mode", None)
    in_maps = _prep_core_inputs(image, W_proj, b_proj)
    nc = _get_nc()
    res = run_bass_kernel_spmd(nc, in_maps, core_ids=list(range(NCORES)),
                               **spmd_kwargs)
    # block layout o{i}[p, n6, j] -> [moff+j, 128*n6+p] -> rows of [M, D]
    outs = []
    for c in range(NCORES):
        rows = np.empty((M, D), dtype=np.float32)
        for i, mw in enumerate(MBLOCKS):
            blk = res.results[c][f"o{i}"]  # [128, NC6, mw] bf16
            rows[MOFF[i]:MOFF[i] + mw] = (
                blk.transpose(2, 1, 0).reshape(mw, D).astype(np.float32))
        outs.append(rows.reshape(BPC, NPATCH, D))
    full = np.concatenate(outs, axis=0)
    return full, res


def kernel(image, W_proj, b_proj):
    full, _ = _run(image, W_proj, b_proj)
    return full
